# revision 9
# baseline (speedup 1.0000x reference)
"""CLUB loss kernel for 8 trn2 NeuronCores.

Math (reference):
    mu     = relu(z_c @ W1m + b1m) @ W2m + b2m
    logvar = tanh(relu(z_c @ W1l + b1l) @ W2l + b2l)
    iv     = 0.5 * exp(-logvar)
    term   = iv * [ 2*mu*(z_d - Ezd) + (Ezd2 - z_d^2) ]     (mu^2 cancels)
    mi     = mean_i sum_d term

Decomposition (iv' = exp(-logvar) = 2*iv):
    N*mi = (s1 - s2/2) + sum_d (Q_d/N)*(A_d/2) - sum_d (P_d/N)*B_d
      s1 = sum iv'*mu*z_d      s2 = sum iv'*z_d^2
      A  = sum_i iv'           B  = sum_i iv'*mu   (per-d vectors)
      P  = sum_i z_d           Q  = sum_i z_d^2
All per-core partials are exact in the row-sharded setting; the host does the
tiny O(D) combine in float64, so no device collective is needed.

Sharding: data-parallel over N (2048 rows/core), weights replicated.
On-chip layout is feature-major ([d, row]); z_c/z_d are transposed through the
PE (identity matmul) so every matmul streams with the natural weight layout.
Matmul operands are fp16 (1 cyc/row on trn2 PE vs 4 for fp32); everything
else stays fp32.
"""

import sys

if "/opt/trn_rl_repo" not in sys.path:
    sys.path.insert(0, "/opt/trn_rl_repo")

import numpy as np

import concourse.bacc as bacc
import concourse.mybir as mybir
import concourse.tile as tile
from concourse.bass import ts
from concourse.bass_utils import run_bass_kernel_spmd

N, DC, H, DD = 16384, 1024, 1024, 1024
NCORES = 8
R = N // NCORES          # rows per core
F = 512                  # row-block (matmul moving dim / PSUM bank)
NB = R // F              # row blocks per core
KC, MC, CC = DC // 128, H // 128, DD // 128
NIDX = NB * CC           # accumulator columns per quantity

F32 = mybir.dt.float32
F16 = mybir.dt.float16
AF = mybir.ActivationFunctionType
OP = mybir.AluOpType

_CACHE = {}


def _build(rows=R):
    nb = rows // F
    nidx = nb * CC
    nc = bacc.Bacc("TRN2", num_devices=NCORES)

    zc = nc.declare_dram_parameter("zc", [rows, DC], F16, isOutput=False)
    zd = nc.declare_dram_parameter("zd", [rows, DD], F32, isOutput=False)
    w = {
        name: nc.declare_dram_parameter(name, [1024, 1024], F16, isOutput=False)
        for name in ("w1m", "w2m", "w1l", "w2l")
    }
    bias = {
        name: nc.declare_dram_parameter(name, [128, 8], F32, isOutput=False)
        for name in ("b1m", "b2m", "b1l", "b2l")
    }
    ident_in = nc.declare_dram_parameter("ident", [128, 128], F32, isOutput=False)
    # acc_act: A cols [0,NIDX), P cols [NIDX,2*NIDX)  (written by ScalarE)
    # acc_dve: B, Q, s1, s2 at col offsets 0/1/2/3 * NIDX (written by VectorE)
    out_act = nc.declare_dram_parameter("acc_act", [128, 2 * nidx], F32, isOutput=True)
    out_dve = nc.declare_dram_parameter("acc_dve", [128, 4 * nidx], F32, isOutput=True)

    from contextlib import ExitStack

    with tile.TileContext(nc) as tc, ExitStack() as es:
        wpool = es.enter_context(tc.tile_pool(name="wpool", bufs=1))
        cpool = es.enter_context(tc.tile_pool(name="cpool", bufs=1))
        stage = es.enter_context(tc.tile_pool(name="stage", bufs=5))
        zct_p = es.enter_context(tc.tile_pool(name="zct", bufs=2))
        zdt_p = es.enter_context(tc.tile_pool(name="zdt", bufs=2))
        h_p = es.enter_context(tc.tile_pool(name="hp", bufs=1))
        ls_p = es.enter_context(tc.tile_pool(name="ls", bufs=3))
        acc_p = es.enter_context(tc.tile_pool(name="accp", bufs=1))
        tp_ps = es.enter_context(tc.tile_pool(name="tpps", bufs=2, space="PSUM"))
        mm_ps = es.enter_context(tc.tile_pool(name="mmps", bufs=4, space="PSUM"))

        # ---- constants / weights (persist whole kernel) ----
        ident = cpool.tile([128, 128], F32, tag="ident")
        nc.sync.dma_start(ident[:], ident_in[:])
        ident16 = cpool.tile([128, 128], F16, tag="ident16")
        nc.scalar.activation(ident16[:], ident[:], AF.Copy)
        bt = {}
        for name in bias:
            bt[name] = cpool.tile([128, 8], F32, tag=f"t_{name}", name=f"t_{name}")
            nc.sync.dma_start(bt[name][:], bias[name][:])
        # stage block 0 inputs before the (large) weight DMAs so the PE's
        # first transposes aren't queued behind them; weights follow in
        # first-use order.
        stage0 = {"zc": [], "zd": []}
        wt = {}

        def load_w(name):
            for k in range(KC):
                t = wpool.tile([128, 1024], F16, tag=f"t_{name}_{k}", name=f"t_{name}_{k}")
                nc.sync.dma_start(t[:], w[name][ts(k, 128), :])
                wt[(name, k)] = t

        for rc in range(4):
            t = stage.tile([128, DC], F16, tag="zc_st", name=f"zc_st_0_{rc}")
            nc.sync.dma_start(t[:], zc[ts(rc, 128), :])
            stage0["zc"].append(t)
        load_w("w1m")
        for rc in range(4):
            t = stage.tile([128, DD], F32, tag="zd_st", name=f"zd_st_0_{rc}")
            nc.sync.dma_start(t[:], zd[ts(rc, 128), :])
            stage0["zd"].append(t)
        load_w("w1l")
        load_w("w2l")
        load_w("w2m")

        acc_a = acc_p.tile([128, 2 * nidx], F32, tag="acc_a")
        acc_d = acc_p.tile([128, 4 * nidx], F32, tag="acc_d")

        for b in range(nb):
            idx0 = b * CC

            # ---- stage + transpose z_c and z_d for this row block ----
            if b == 0:
                zc_st = stage0["zc"]
                zd_st = stage0["zd"]
            else:
                zc_st = []
                zd_st = []
                for rc in range(4):
                    t = stage.tile([128, DC], F16, tag="zc_st", name=f"zc_st_{b}_{rc}")
                    nc.sync.dma_start(t[:], zc[ts(4 * b + rc, 128), :])
                    zc_st.append(t)
                    t = stage.tile([128, DD], F32, tag="zd_st", name=f"zd_st_{b}_{rc}")
                    nc.sync.dma_start(t[:], zd[ts(4 * b + rc, 128), :])
                    zd_st.append(t)

            zct = []
            for k in range(KC):
                ps = tp_ps.tile([128, F], F16, tag="tp16")
                for rc in range(4):
                    nc.tensor.transpose(
                        ps[:, ts(rc, 128)], zc_st[rc][:, ts(k, 128)], ident16[:]
                    )
                t = zct_p.tile([128, F], F16, tag=f"zct{k}", name=f"zct_{b}_{k}")
                nc.scalar.activation(t[:], ps[:], AF.Copy)
                zct.append(t)

            def l1(wname, bname):
                hs = []
                for m in range(MC):
                    ps = mm_ps.tile([128, F], F32, tag="mm")
                    for k in range(KC):
                        nc.tensor.matmul(
                            ps[:], wt[(wname, k)][:, ts(m, 128)], zct[k][:],
                            start=(k == 0), stop=(k == KC - 1),
                        )
                    ht = h_p.tile([128, F], F16, tag=f"h_{wname}_{m}", name=f"h_{wname}_{b}_{m}")
                    nc.scalar.activation(
                        ht[:], ps[:], AF.Relu, bias=bt[bname][:, m : m + 1]
                    )
                    hs.append(ht)
                return hs

            # L1(mu) fills the PE while z_d staging/weights stream in
            h_mu = l1("w1m", "b1m")

            zdt = []
            for k in range(KC):
                ps = tp_ps.tile([128, F], F32, tag="tp")
                for rc in range(4):
                    nc.tensor.transpose(
                        ps[:, ts(rc, 128)], zd_st[rc][:, ts(k, 128)], ident[:]
                    )
                t = zdt_p.tile([128, F], F32, tag=f"zdt{k}", name=f"zdt_{b}_{k}")
                nc.scalar.activation(
                    t[:], ps[:], AF.Copy,
                    accum_out=acc_a[:, nidx + idx0 + k : nidx + idx0 + k + 1],
                )
                zdt.append(t)

            h_lv = l1("w1l", "b1l")

            # ---- layer 2 + loss, per output-feature chunk c ----
            for c in range(CC):
                i1 = idx0 + c

                ps_lv = mm_ps.tile([128, F], F32, tag="mm")
                for m in range(MC):
                    nc.tensor.matmul(
                        ps_lv[:], wt[("w2l", m)][:, ts(c, 128)], h_lv[m][:],
                        start=(m == 0), stop=(m == MC - 1),
                    )
                lg = ls_p.tile([128, F], F32, tag="lg")
                nc.scalar.activation(
                    lg[:], ps_lv[:], AF.Tanh, bias=bt["b2l"][:, c : c + 1]
                )
                iv = ls_p.tile([128, F], F32, tag="iv")
                nc.scalar.activation(
                    iv[:], lg[:], AF.Exp, scale=-1.0,
                    accum_out=acc_a[:, i1 : i1 + 1],
                )

                ps_mu = mm_ps.tile([128, F], F32, tag="mm")
                for m in range(MC):
                    nc.tensor.matmul(
                        ps_mu[:], wt[("w2m", m)][:, ts(c, 128)], h_mu[m][:],
                        start=(m == 0), stop=(m == MC - 1),
                    )
                # g = (mu_psum + b2m) * iv ; B += sum(g)
                g = ls_p.tile([128, F], F32, tag="g")
                nc.vector.scalar_tensor_tensor(
                    g[:], ps_mu[:], bt["b2m"][:, c : c + 1], iv[:],
                    op0=OP.add, op1=OP.mult,
                    accum_out=acc_d[:, i1 : i1 + 1],
                )
                scr = ls_p.tile([128, F], F32, tag="scr")
                # s1 += sum(g * zd)
                nc.vector.scalar_tensor_tensor(
                    scr[:], g[:], 0.0, zdt[c][:], op0=OP.add, op1=OP.mult,
                    accum_out=acc_d[:, 2 * nidx + i1 : 2 * nidx + i1 + 1],
                )
                # r = iv * zd ; s2 += sum(r * zd)
                r = ls_p.tile([128, F], F32, tag="r")
                nc.vector.tensor_tensor(r[:], iv[:], zdt[c][:], OP.mult)
                nc.vector.scalar_tensor_tensor(
                    scr[:], r[:], 0.0, zdt[c][:], op0=OP.add, op1=OP.mult,
                    accum_out=acc_d[:, 3 * nidx + i1 : 3 * nidx + i1 + 1],
                )
                # Q += sum(zd * zd)
                nc.vector.scalar_tensor_tensor(
                    scr[:], zdt[c][:], 0.0, zdt[c][:], op0=OP.add, op1=OP.mult,
                    accum_out=acc_d[:, nidx + i1 : nidx + i1 + 1],
                )

        nc.sync.dma_start(out_act[:], acc_a[:])
        nc.sync.dma_start(out_dve[:], acc_d[:])

    nc.compile()
    return nc


def kernel(z_c, z_d, W1_mu, b1_mu, W2_mu, b2_mu, W1_lv, b1_lv, W2_lv, b2_lv):
    if "nc" not in _CACHE:
        _CACHE["nc"] = _build()
    nc = _CACHE["nc"]

    common = {
        "w1m": np.ascontiguousarray(W1_mu.astype(np.float16)),
        "w2m": np.ascontiguousarray(W2_mu.astype(np.float16)),
        "w1l": np.ascontiguousarray(W1_lv.astype(np.float16)),
        "w2l": np.ascontiguousarray(W2_lv.astype(np.float16)),
        "b1m": np.ascontiguousarray(b1_mu.reshape(8, 128).T.astype(np.float32)),
        "b2m": np.ascontiguousarray(b2_mu.reshape(8, 128).T.astype(np.float32)),
        "b1l": np.ascontiguousarray(b1_lv.reshape(8, 128).T.astype(np.float32)),
        "b2l": np.ascontiguousarray(b2_lv.reshape(8, 128).T.astype(np.float32)),
        "ident": np.eye(128, dtype=np.float32),
    }
    z_c = np.asarray(z_c).astype(np.float16)
    z_d = np.asarray(z_d, dtype=np.float32)
    in_maps = [
        {
            "zc": np.ascontiguousarray(z_c[i * R : (i + 1) * R]),
            "zd": np.ascontiguousarray(z_d[i * R : (i + 1) * R]),
            **common,
        }
        for i in range(NCORES)
    ]

    res = run_bass_kernel_spmd(nc, in_maps, list(range(NCORES)))

    A = np.zeros(DD, dtype=np.float64)
    B = np.zeros(DD, dtype=np.float64)
    P = np.zeros(DD, dtype=np.float64)
    Q = np.zeros(DD, dtype=np.float64)
    s1 = 0.0
    s2 = 0.0

    def vec(cols):  # [128, NIDX] partials -> [DD] summed over blocks
        v = cols.astype(np.float64).reshape(128, NB, CC).sum(axis=1)  # [p, c]
        return v.T.reshape(DD)  # d = c*128 + p

    for i in range(NCORES):
        oa = res.results[i]["acc_act"]
        od = res.results[i]["acc_dve"]
        A += vec(oa[:, :NIDX])
        P += vec(oa[:, NIDX:])
        B += vec(od[:, :NIDX])
        Q += vec(od[:, NIDX : 2 * NIDX])
        s1 += od[:, 2 * NIDX : 3 * NIDX].astype(np.float64).sum()
        s2 += od[:, 3 * NIDX :].astype(np.float64).sum()

    total = (s1 - 0.5 * s2) + float(Q @ A) / (2.0 * N) - float(P @ B) / N
    return np.asarray(total / N, dtype=np.float32)
